# revision 1
# baseline (speedup 1.0000x reference)
"""DrugGCN kernel entry point (dev version: imports sibling modules).

The graded kernel.py is generated by make_kernel.py as a self-contained file.
"""
import time
import numpy as np

from bench_util import patch_tile_drain
patch_tile_drain()

import gcn_build as G
from runner import SpmdRunner

_CACHE = {}
_TIMING = {"exec_ns": float("nan")}

N_NODES = 50000
N_CORES = 8
WINDOW = 64
CAP = 1216


def _get_runner(edge_key, edge_index):
    if edge_key in _CACHE:
        return _CACHE[edge_key]
    P = G.make_plan(N_NODES, N_CORES, window=WINDOW, cap=CAP)
    per_core = G.preprocess(edge_index, P)
    nc = G.build_kernel(P)
    r = SpmdRunner(nc, N_CORES)
    _CACHE[edge_key] = (P, per_core, r)
    return _CACHE[edge_key]


def kernel(**inputs):
    x = np.asarray(inputs["x"], np.float32)
    edge_index = np.asarray(inputs["edge_index"])
    assert x.shape == (N_NODES, 64), x.shape
    ek = hash(edge_index.tobytes())
    P, per_core, r = _get_runner(ek, edge_index)
    in_maps = G.make_in_maps(inputs, P, per_core)
    args = r.prepare(in_maps)
    out = r(args)
    # wall-clock timing over repeat calls (device-resident args)
    times = []
    for _ in range(5):
        t0 = time.perf_counter()
        out = r(args)
        times.append(time.perf_counter() - t0)
    _TIMING["exec_ns"] = min(times) * 1e9
    _TIMING["wall_times_ms"] = [t * 1e3 for t in times]
    results = r.unpack(out)
    return G.assemble_output(results, P)


def _timing_info():
    return _TIMING
